# revision 2
# baseline (speedup 1.0000x reference)
"""DeepSeekMoE Trainium2 kernel: 8-way expert-parallel, host-routed dispatch.

Layout notes (per core e):
  - routed expert e computes only its assigned tokens (gathered, zero-padded
    to `cap`), fed transposed as xet [D, cap] so the contraction dim D lands
    on SBUF partitions with no on-device transposes.
  - the shared SwiGLU is tensor-parallel over the inter dim: each core owns a
    352-wide slice (zero-padded to 384) of sw1/sw3/sw2 and produces a partial
    y over all tokens.
  - gate runs on-device for the gathered tokens; gate_w arrives with the
    core's own expert permuted to column 0 so one SPMD program serves all 8.
  - all matmuls run as float32r (TF32-class precision at full bf16 rate).
Host combines: y = sum_e ypart_e; y[idx_e] += yrouted_e[:cnt_e].
"""
import numpy as np

import concourse.bass as bass
import concourse.mybir as mybir
import concourse.tile as tile
from concourse import bacc
from concourse.bass_utils import run_bass_kernel_spmd

D = 2048
F = 1408
E = 8
TOPK = 2
FS_FULL = 2816        # F * N_SHARED
FSL = 352             # per-core shared slice (FS_FULL / 8)
FSP = 384             # padded to 3*128
NT = 2048             # tokens (2*1024)
P = 128
DK = D // P           # 16
FK = F // P           # 11
SK = FSP // P         # 3
F32 = mybir.dt.float32
F32R = mybir.dt.float32r
SILU = mybir.ActivationFunctionType.Silu
EXP = mybir.ActivationFunctionType.Exp

_nc_cache: dict[int, object] = {}


def _chunks(total, step):
    out = []
    o = 0
    while o < total:
        out.append((o, min(step, total - o)))
        o += step
    return out


def _build(cap: int):
    """Build the SPMD program for per-expert token capacity `cap` (mult of 128)."""
    from contextlib import ExitStack
    capm = cap // P
    # free-dim chunks >=256 keep float32r at full rate
    cap_chunks = []
    rem = cap
    while rem > 0:
        take = 512 if rem >= 768 or rem == 512 else (rem if rem <= 512 else rem - 256)
        cap_chunks.append(take)
        rem -= take
    assert sum(cap_chunks) == cap and all(c >= 256 or cap < 256 for c in cap_chunks)

    nc = bacc.Bacc("TRN2", target_bir_lowering=False)
    xet = nc.declare_dram_parameter("xet", [D, cap], F32R, isOutput=False)
    gwe = nc.declare_dram_parameter("gwe", [D, E], F32R, isOutput=False)
    rw1 = nc.declare_dram_parameter("rw1", [D, F], F32R, isOutput=False)
    rw3 = nc.declare_dram_parameter("rw3", [D, F], F32R, isOutput=False)
    rw2 = nc.declare_dram_parameter("rw2", [F, D], F32R, isOutput=False)
    swa = nc.declare_dram_parameter("swa", [D, FSP], F32R, isOutput=False)
    swb = nc.declare_dram_parameter("swb", [D, FSP], F32R, isOutput=False)
    swc = nc.declare_dram_parameter("swc", [FSP, D], F32R, isOutput=False)
    xt = nc.declare_dram_parameter("xt", [D, NT], F32R, isOutput=False)
    yrouted = nc.declare_dram_parameter("yrouted", [cap, D], F32, isOutput=True)
    ypart = nc.declare_dram_parameter("ypart", [NT, D], F32, isOutput=True)

    xet_r = xet.rearrange("(ko p) c -> ko p c", p=P)
    xt_r = xt.rearrange("(ko p) c -> ko p c", p=P)
    quarters = _chunks(NT, 512)    # shared layer-1 token chunks

    with tile.TileContext(nc) as tc, ExitStack() as es:
        gate_pool = es.enter_context(tc.tile_pool(name="gate", bufs=1))
        swab_pool = es.enter_context(tc.tile_pool(name="swab", bufs=1))
        GW = gate_pool.tile([P, DK, E], F32R)
        WCOL = gate_pool.tile([P, capm], F32)
        SWA = swab_pool.tile([P, DK, FSP], F32R)
        SWB = swab_pool.tile([P, DK, FSP], F32R)

        def gate_col(mt, XET, psum, stage):
            ps = psum.tile([P, E], F32, tag="gate_ps")
            for k in range(DK):
                nc.tensor.matmul(
                    ps[:], XET[:, k, bass.ts(mt, P)], GW[:, k],
                    start=(k == 0), stop=(k == DK - 1))
            nmax = stage.tile([P, 1], F32, tag="gate_nmax")
            nc.vector.reduce_max(nmax[:], ps[:], axis=mybir.AxisListType.X,
                                 negate=True)
            es_t = stage.tile([P, E], F32, tag="gate_es")
            nc.scalar.activation(es_t[:], ps[:], EXP, bias=nmax[:])
            ssum = stage.tile([P, 1], F32, tag="gate_sum")
            nc.vector.reduce_sum(ssum[:], es_t[:], axis=mybir.AxisListType.X)
            rec = stage.tile([P, 1], F32, tag="gate_rec")
            nc.vector.reciprocal(rec[:], ssum[:])
            nc.vector.tensor_mul(WCOL[:, mt:mt + 1], es_t[:, 0:1], rec[:])

        # ---- routed phase ----
        with tc.tile_pool(name="gt", bufs=1) as gt_pool, \
             tc.tile_pool(name="stage_rt", bufs=3) as stage, \
             tc.tile_pool(name="psum_rt", bufs=2, space="PSUM") as psum:
            GT = gt_pool.tile([P, FK, cap], F32R)

            # layer 1 (gate interleaved): gT = silu(x@w1) * (x@w3)
            with tc.tile_pool(name="rt1x", bufs=1) as xet_pool, \
                 tc.tile_pool(name="rt1w", bufs=2) as wpool1:
                nc.sync.dma_start(GW[:], gwe.rearrange("(ko p) c -> p ko c", p=P))
                XET = xet_pool.tile([P, DK, cap], F32R)
                # per-k row loads: the gate and layer-1 k-loops pipeline
                # behind the stream, chunk by chunk
                for k in range(4):
                    nc.sync.dma_start(XET[:, k], xet_r[k])
                w1c0 = wpool1.tile([P, DK, P], F32R, tag="w1c")
                nc.sync.dma_start(
                    w1c0[:], rw1[:, 0:P].rearrange("(ko p) c -> p ko c", p=P))
                w3c0 = wpool1.tile([P, DK, P], F32R, tag="w3c")
                nc.sync.dma_start(
                    w3c0[:], rw3[:, 0:P].rearrange("(ko p) c -> p ko c", p=P))
                for k in range(4, DK):
                    nc.sync.dma_start(XET[:, k], xet_r[k])
                for mt in range(capm):
                    gate_col(mt, XET, psum, stage)

                for m in range(FK):
                    if m == 0:
                        w1c, w3c = w1c0, w3c0
                    else:
                        w1c = wpool1.tile([P, DK, P], F32R, tag="w1c")
                        nc.sync.dma_start(
                            w1c[:],
                            rw1[:, bass.ts(m, P)].rearrange(
                                "(ko p) c -> p ko c", p=P))
                        w3c = wpool1.tile([P, DK, P], F32R, tag="w3c")
                        nc.sync.dma_start(
                            w3c[:],
                            rw3[:, bass.ts(m, P)].rearrange(
                                "(ko p) c -> p ko c", p=P))
                    if m == 8:
                        # shared-phase residents slot into the tail of the
                        # layer-1 weight stream, where the pipe runs ahead
                        nc.sync.dma_start(
                            SWA[:], swa.rearrange("(ko p) c -> p ko c", p=P))
                        nc.sync.dma_start(
                            SWB[:], swb.rearrange("(ko p) c -> p ko c", p=P))
                    n0 = 0
                    for nw in cap_chunks:
                        psa = psum.tile([P, 512], F32, tag="rt1a")
                        for k in range(DK):
                            nc.tensor.matmul(
                                psa[:, :nw], w1c[:, k], XET[:, k, n0:n0 + nw],
                                start=(k == 0), stop=(k == DK - 1))
                        sa = stage.tile([P, 512], F32, tag="rt1_silu")
                        nc.scalar.activation(sa[:, :nw], psa[:, :nw], SILU)
                        psb = psum.tile([P, 512], F32, tag="rt1b")
                        for k in range(DK):
                            nc.tensor.matmul(
                                psb[:, :nw], w3c[:, k], XET[:, k, n0:n0 + nw],
                                start=(k == 0), stop=(k == DK - 1))
                        nc.vector.tensor_mul(GT[:, m, n0:n0 + nw], sa[:, :nw],
                                             psb[:, :nw])
                        n0 += nw

            # layer 2: y_e[t, :] = w_t * (gT_t @ rw2)
            sh_res = es.enter_context(tc.tile_pool(name="sh_res", bufs=1, side="right"))
            SWC = sh_res.tile([P, SK, D], F32R)
            nc.scalar.dma_start(SWC[:], swc.rearrange("(ko p) c -> p ko c", p=P))
            GST = sh_res.tile([P, SK, NT], F32R)
            with tc.tile_pool(name="rt2w", bufs=2, side="right") as wpool2:
                for (n0, nw) in _chunks(D, 512):
                    w2n = wpool2.tile([P, FK, 512], F32R, tag="w2n")
                    nc.sync.dma_start(
                        w2n[:],
                        rw2[:, n0:n0 + nw].rearrange("(ko p) c -> p ko c", p=P))
                    for mt in range(capm):
                        ps = psum.tile([P, 512], F32, tag="rt2")
                        for k in range(FK):
                            nc.tensor.matmul(
                                ps[:], GT[:, k, bass.ts(mt, P)], w2n[:, k],
                                start=(k == 0), stop=(k == FK - 1))
                        ot = stage.tile([P, 512], F32, tag="rt2_out")
                        nc.vector.tensor_scalar_mul(ot[:], ps[:],
                                                    WCOL[:, mt:mt + 1])
                        nc.scalar.dma_start(yrouted[bass.ts(mt, P), n0:n0 + nw],
                                            ot[:])

        # ---- shared expert (TP slice): layer 1 + layer 2 per token chunk ----
        with tc.tile_pool(name="sh_xt", bufs=2, side="right") as xtq_pool, \
             tc.tile_pool(name="stage_sh", bufs=3, side="right") as stage_sh, \
             tc.tile_pool(name="psum_sh", bufs=2, space="PSUM") as psum_sh:
            for qi, (q0, qw) in enumerate(quarters):
                XTQ = xtq_pool.tile([P, DK, 512], F32R, tag="xtq")
                for k in range(DK):
                    nc.sync.dma_start(XTQ[:, k, :qw], xt_r[k, :, q0:q0 + qw])
                for m in range(SK):
                    psa = psum_sh.tile([P, 512], F32, tag="sh1a")
                    for k in range(DK):
                        nc.tensor.matmul(
                            psa[:, :qw], SWA[:, k, bass.ts(m, P)], XTQ[:, k, :qw],
                            start=(k == 0), stop=(k == DK - 1))
                    sa = stage_sh.tile([P, 512], F32, tag="sh1_silu")
                    nc.scalar.activation(sa[:, :qw], psa[:, :qw], SILU)
                    psb = psum_sh.tile([P, 512], F32, tag="sh1b")
                    for k in range(DK):
                        nc.tensor.matmul(
                            psb[:, :qw], SWB[:, k, bass.ts(m, P)], XTQ[:, k, :qw],
                            start=(k == 0), stop=(k == DK - 1))
                    nc.vector.tensor_mul(GST[:, m, q0:q0 + qw], sa[:, :qw],
                                         psb[:, :qw])

                # layer 2 for this chunk's tokens: ypart[t, :] = gsT_t @ swc
                for mt in range(q0 // P, (q0 + qw) // P):
                    for (n0, nw) in _chunks(D, 512):
                        ps = psum_sh.tile([P, 512], F32, tag="sh2")
                        for k in range(SK):
                            nc.tensor.matmul(
                                ps[:], GST[:, k, bass.ts(mt, P)],
                                SWC[:, k, n0:n0 + nw],
                                start=(k == 0), stop=(k == SK - 1))
                        ot = stage_sh.tile([P, 512], F32, tag="sh2_out")
                        nc.vector.tensor_copy(ot[:], ps[:])
                        nc.scalar.dma_start(ypart[bass.ts(mt, P), n0:n0 + nw],
                                            ot[:])

    nc.compile()
    return nc

def _route(xf: np.ndarray, gate_w: np.ndarray):
    logits = xf @ gate_w
    m = logits.max(-1, keepdims=True)
    ex = np.exp(logits - m)
    scores = ex / ex.sum(-1, keepdims=True)
    top2 = np.argsort(-scores, axis=-1)[:, :TOPK]
    return top2


LAST_RESULTS = None


def _prep(x, gate_w, sw1, sw2, sw3, rw1, rw2, rw3):
    x = np.asarray(x, dtype=np.float32)
    xf = np.ascontiguousarray(x.reshape(-1, D))
    gate_w = np.asarray(gate_w, dtype=np.float32)
    top2 = _route(xf, gate_w)

    idx = [np.where((top2 == e).any(axis=1))[0] for e in range(E)]
    maxcnt = max(len(i) for i in idx)
    cap = max(512, -(-maxcnt // P) * P)

    xtf = np.ascontiguousarray(xf.T)  # [D, NT]
    sw1 = np.asarray(sw1, dtype=np.float32)
    sw2 = np.asarray(sw2, dtype=np.float32)
    sw3 = np.asarray(sw3, dtype=np.float32)
    rw1 = np.asarray(rw1, dtype=np.float32)
    rw2 = np.asarray(rw2, dtype=np.float32)
    rw3 = np.asarray(rw3, dtype=np.float32)

    in_maps = []
    for e in range(E):
        ie = idx[e]
        xet = np.zeros((D, cap), dtype=np.float32)
        xet[:, :len(ie)] = xf[ie].T
        perm = [e] + [j for j in range(E) if j != e]
        gwe = np.ascontiguousarray(gate_w[:, perm])
        swa = np.zeros((D, FSP), dtype=np.float32)
        swa[:, :FSL] = sw1[:, e * FSL:(e + 1) * FSL]
        swb = np.zeros((D, FSP), dtype=np.float32)
        swb[:, :FSL] = sw3[:, e * FSL:(e + 1) * FSL]
        swc = np.zeros((FSP, D), dtype=np.float32)
        swc[:FSL] = sw2[e * FSL:(e + 1) * FSL]
        in_maps.append({
            "xet": xet, "gwe": gwe,
            "rw1": np.ascontiguousarray(rw1[e]),
            "rw3": np.ascontiguousarray(rw3[e]),
            "rw2": np.ascontiguousarray(rw2[e]),
            "swa": swa, "swb": swb, "swc": swc, "xt": xtf,
        })
    return idx, cap, in_maps


def build_in_maps(inputs, cap=None):
    _idx, c, in_maps = _prep(**inputs)
    assert cap is None or c == cap, (c, cap)
    return in_maps


def kernel(x, gate_w, sw1, sw2, sw3, rw1, rw2, rw3, _trace=False):
    x = np.asarray(x, dtype=np.float32)
    B, T, _ = x.shape
    idx, cap, in_maps = _prep(x, gate_w, sw1, sw2, sw3, rw1, rw2, rw3)
    if cap not in _nc_cache:
        _nc_cache[cap] = _build(cap)
    nc = _nc_cache[cap]

    res = run_bass_kernel_spmd(nc, in_maps, list(range(E)), trace=_trace)
    global LAST_RESULTS
    LAST_RESULTS = res

    y = res.results[0]["ypart"].astype(np.float32).copy()
    for e in range(1, E):
        y += res.results[e]["ypart"]
    for e in range(E):
        ie = idx[e]
        y[ie] += res.results[e]["yrouted"][:len(ie)]
    return y.reshape(B, T, D)

